# revision 31
# baseline (speedup 1.0000x reference)
"""Causal single-head attention (B=4, S=2048, D=DK=1024) on 8 trn2 NeuronCores.

Sharding: data-parallel over batch x interleaved q-blocks. Core c handles
batch b=c//2, parity p=c%2, owning the 8 q-blocks {2j+p : j in 0..7} (128 rows
each). One uniform SPMD program runs on all 8 cores; per-core differences are
carried entirely by the input data (host-side column permutation + mask tiles).

Math per core (weight-folded; W_QK = W_Q W_K^T folded on the host):
    G^T = W_QK^T X_q^T                [d, 1024]
    S   = G X_ctx^T   (causal window, compact 2-region layout)
    A   = softmax(S/32 with -1e9 mask pre-scale)
    P   = A X_ctx     (bf16)
    out = P W_V       (bf16, then scatter rows back on host)

vs the previous version: all 136 PE transposes replaced by 16 XBAR DMA
transposes (one per A / one per P per q-tile); G runs dc-outer so the first
matmul needs only the first wqk chunk; the causal mask is added by an
identity-matmul accumulation into the scores PSUM; W_V / P / out run bf16;
O-stage is software-pipelined one q-tile behind to fill softmax bubbles; the
j-loop runs [1..7, 0] so the final tile has the shortest tail.
"""

import numpy as np
import ml_dtypes

B, S, D = 4, 2048, 1024
P = 128               # partitions
NJ = 8                # q-tiles per core
NCORES = 8
MASK_FILL = -1.0e9

_cache = {}


def _chunks(hw):
    """Split a region of width hw into <=512-wide balanced pieces."""
    if hw <= 512:
        return [(0, hw)]
    half = hw // 2
    return [(0, half), (half, hw - half)]


def _build_program():
    from contextlib import ExitStack
    import concourse.bass as bass
    import concourse.bacc as bacc
    import concourse.tile as tile
    import concourse.mybir as mybir
    from concourse import masks

    f32 = mybir.dt.float32
    f32r = mybir.dt.float32r
    bf16 = mybir.dt.bfloat16
    Exp = mybir.ActivationFunctionType.Exp
    Copy = mybir.ActivationFunctionType.Copy
    AX = mybir.AxisListType.X
    ts = bass.ts

    nc = bacc.Bacc("TRN2", target_bir_lowering=False, debug=False,
                   enable_asserts=False)

    xct_d = nc.dram_tensor("xct", [D, S], f32r, kind="ExternalInput").ap()
    xc_d = nc.dram_tensor("xc", [S, D], bf16, kind="ExternalInput").ap()
    wqk_d = nc.dram_tensor("wqk", [D, D], f32r, kind="ExternalInput").ap()
    wv_d = nc.dram_tensor("wv", [D, D], bf16, kind="ExternalInput").ap()
    madd_d = nc.dram_tensor("madd", [P, NJ * 2 * P], bf16,
                            kind="ExternalInput").ap()
    out_d = nc.dram_tensor("out", [NJ * P, D], bf16, kind="ExternalOutput").ap()

    xct_r = xct_d.rearrange("(c p) k -> c p k", p=P)    # [8, 128, 2048]
    xc_r = xc_d.rearrange("(c p) d -> c p d", p=P)      # [16, 128, 1024]
    wqk_r = wqk_d.rearrange("(c p) n -> c p n", p=P)
    wv_r = wv_d.rearrange("(c p) n -> c p n", p=P)
    madd_r = madd_d.rearrange("p (j c) -> p j c", j=NJ)  # [128, 8, 256]

    with tile.TileContext(nc) as tc, ExitStack() as es:
        # ---- persistent pools -------------------------------------------
        perm = es.enter_context(tc.tile_pool(name="perm", bufs=1))
        xct_sb = perm.tile([P, 8, S], f32r)       # X_ctx^T  64KB/part
        xc_sb = perm.tile([P, 16, D], bf16)       # X_ctx (perm rows) 32KB/part
        gt_sb = perm.tile([P, 8, 1024], f32r)     # G^T 32KB/part
        madd_sb = perm.tile([P, NJ, 2 * P], bf16)  # 4KB/part
        ident_b = perm.tile([P, P], bf16)
        masks.make_identity(nc, ident_b[:])

        # phase-D pools allocated before the G scope so early q-tiles can
        # overlap with the tail of G (no release-barrier dependency).
        statp = es.enter_context(tc.tile_pool(name="stats", bufs=4))
        earlyp = es.enter_context(tc.tile_pool(name="early", bufs=2))
        attntp = es.enter_context(tc.tile_pool(name="attnt", bufs=2))
        workp = es.enter_context(tc.tile_pool(name="work", bufs=2))
        # score/P PSUM pools pre-allocated (banks 0-3) so S j1..j3 run
        # concurrently with G's qh1 passes without any PSUM reuse hazard.
        spsp = es.enter_context(tc.tile_pool(name="sps", bufs=2, space="PSUM"))
        ppp = es.enter_context(tc.tile_pool(name="pps", bufs=2, space="PSUM"))

        with tc.tile_pool(name="wqk", bufs=1) as wqkp, \
             tc.tile_pool(name="gps", bufs=4, space="PSUM") as gps:
            wqk_sb = wqkp.tile([P, 8, 1024], f32r)
            # HAM warm-up: dependency-free matmuls fill the PE-idle
            # window while the first input chunks stream in.
            warm = gps.tile([P, 512], f32, tag="gps", name="warmup")
            for _ in range(36):
                nc.tensor.matmul(warm[:, 0:P], ident_b[:], ident_b[:])
            # DMA issue order == first-use order on the sync ring.  wqk is
            # split by dt-half so G's first 4-bank pass streams on a
            # 0.5MB-granular feed; bulk tail uses few large DMAs.
            xct_3d = xct_d.rearrange("(c p) k -> p c k", p=P)
            wqk_4d = wqk_d.rearrange("(c p) (h n) -> p c h n", p=P, h=2)
            for c0 in range(0, 8, 2):
                nc.sync.dma_start(wqk_sb[:, c0:c0 + 2, 0:512],
                                  wqk_4d[:, c0:c0 + 2, 0, :])
                nc.sync.dma_start(xct_sb[:, c0:c0 + 2, 0:512],
                                  xct_3d[:, c0:c0 + 2, 0:512])
            nc.scalar.dma_start(madd_sb[:], madd_r)
            for c0 in range(0, 8, 4):
                nc.sync.dma_start(wqk_sb[:, c0:c0 + 4, 512:1024],
                                  wqk_4d[:, c0:c0 + 4, 1, :])
            # region-2 block 8 first so S j1/j2 can overlap G qh1
            nc.sync.dma_start(xct_sb[:, :, 1024:1280],
                              xct_3d[:, :, 1024:1280])
            nc.sync.dma_start(xct_sb[:, :, 512:1024],
                              xct_3d[:, :, 512:1024])
            nc.sync.dma_start(xct_sb[:, :, 1280:1536],
                              xct_3d[:, :, 1280:1536])
            xc_3d = xc_d.rearrange("(c p) d -> p c d", p=P)
            for b0, b1 in ((0, 2), (8, 10)):
                nc.sync.dma_start(xc_sb[:, b0:b1, :], xc_3d[:, b0:b1, :])
            nc.sync.dma_start(xct_sb[:, :, 1536:2048],
                              xct_3d[:, :, 1536:2048])
            for b0, b1 in ((2, 4), (10, 12), (4, 8), (12, 16)):
                nc.sync.dma_start(xc_sb[:, b0:b1, :], xc_3d[:, b0:b1, :])

            # G: four dc-outer passes of 4 dt-tiles each; pass 1 streams on
            # the wqk dt-half feed, later passes run on resident data.
            for qh, dtg in ((0, 0), (0, 4), (512, 0), (512, 4)):
                psl = {dt: gps.tile([P, 512], f32, tag="gps",
                                    name=f"psG{dt}{qh}")
                       for dt in range(dtg, dtg + 4)}
                for dc in range(8):
                    for dt in range(dtg, dtg + 4):
                        nc.tensor.matmul(
                            psl[dt][:], wqk_sb[:, dc, ts(dt, P)],
                            xct_sb[:, dc, qh:qh + 512],
                            start=(dc == 0), stop=(dc == 7))
                for dt in range(dtg, dtg + 4):
                    eng = nc.vector.tensor_copy if dt % 2 else nc.scalar.copy
                    eng(gt_sb[:, dt, qh:qh + 512], psl[dt][:])

        # wv reuses the SBUF freed by wqk; last on the sync ring (fires when
        # the WAR on wqk's space clears, i.e. G done).
        wvp = tc.alloc_tile_pool(name="wv", bufs=1)
        wv_sb = wvp.tile([P, 8, 1024], bf16)
        wv_3d = wv_d.rearrange("(c p) n -> p c n", p=P)
        nc.sync.dma_start(wv_sb[:, 0:4, :], wv_3d[:, 0:4, :])
        nc.sync.dma_start(wv_sb[:, 4:8, :], wv_3d[:, 4:8, :])
        opsp = tc.alloc_tile_pool(name="ops", bufs=2, space="PSUM")
        trp = tc.alloc_tile_pool(name="trp", bufs=2, space="PSUM")

        # ---- phase D: attention per q-tile ------------------------------
        # j order [1..7, 0]: early tiles overlap the tail of G; the last
        # tile (j=0) has the shortest softmax->out tail.  The O-stage runs
        # one tile behind so its matmuls fill the next tile's softmax gap.
        def emit_o(j, p_sb, rcp):
            # P^T here, one tile behind the main loop: its scalar-queue slot
            # lands after exp(j+1)/attnT(j+1), so it never stalls them.  The
            # last tile uses PE transposes to shorten the serial tail.
            pt = workp.tile([P, 8, P], bf16, tag="pt")
            if j == 0:
                for dc in range(8):
                    tp = trp.tile([P, P], bf16, tag="tr")
                    nc.tensor.transpose(tp[:], p_sb[:, ts(dc, P)], ident_b[:])
                    nc.vector.tensor_copy(pt[:, dc, :], tp[:])
            else:
                nc.scalar.dma_start(pt[:], p_sb[:], transpose=True)
            op0 = opsp.tile([P, 512], f32, tag="op", name="op0")
            op1 = opsp.tile([P, 512], f32, tag="op", name="op1")
            for dc in range(8):
                for op, dvh in ((op0, 0), (op1, 512)):
                    nc.tensor.matmul(
                        op[:], pt[:, dc, :], wv_sb[:, dc, dvh:dvh + 512],
                        start=(dc == 0), stop=(dc == 7))
            out_sb = workp.tile([P, 1024], bf16, tag="out", bufs=1)
            nc.scalar.activation(out_sb[:, 0:512], op0[:], Copy, scale=rcp[:])
            nc.vector.tensor_scalar_mul(out_sb[:, 512:1024], op1[:], rcp[:])
            nc.sync.dma_start(out_d[ts(j, P), :], out_sb[:])

        prev_o = None
        for j in (1, 2, 3, 4, 5, 6, 7, 0):
            nk = 2 * j + 2          # 128-wide k-chunks this q-tile
            W = nk * P              # compact context width
            hw = (j + 1) * P        # per-region width

            srow = earlyp.tile([P, 2048], f32, tag="srow")
            mxseg = statp.tile([P, 4], f32, tag="mxseg")
            segs = []
            si = 0
            for ri, (bs, bd) in enumerate(((0, 0), (1024, hw))):
                for off, w in _chunks(hw):
                    ps = spsp.tile([P, 512], f32, tag="ps")
                    last = off + w == hw
                    for dc in range(8):
                        nc.tensor.matmul(
                            ps[:, :w], gt_sb[:, dc, ts(j, P)],
                            xct_sb[:, dc, bs + off:bs + off + w],
                            start=(dc == 0), stop=(dc == 7 and not last))
                    if last:
                        # additive causal mask via identity-matmul accumulate
                        nc.tensor.matmul(
                            ps[:, w - P:w], ident_b[:],
                            madd_sb[:, j, ri * P:(ri + 1) * P],
                            start=False, stop=True)
                    dst = bd + off
                    nc.vector.tensor_copy(srow[:, dst:dst + w], ps[:, :w])
                    nc.vector.reduce_max(mxseg[:, si:si + 1],
                                         srow[:, dst:dst + w], axis=AX)
                    segs.append((dst, w))
                    si += 1
            nmx = statp.tile([P, 1], f32, tag="nmx")
            nc.vector.reduce_max(nmx[:], mxseg[:, :si], axis=AX, negate=True)
            nc.scalar.mul(nmx[:], nmx[:], 1.0 / 32.0)
            seseg = statp.tile([P, 4], f32, tag="seseg")
            attn = earlyp.tile([P, 2048], bf16, tag="attn")
            for k, (dst, w) in enumerate(segs):
                nc.scalar.activation(attn[:, dst:dst + w],
                                     srow[:, dst:dst + w], Exp,
                                     bias=nmx[:], scale=1.0 / 32.0,
                                     accum_out=seseg[:, k:k + 1])
            sumexp = statp.tile([P, 1], f32, tag="se")
            nc.vector.reduce_sum(sumexp[:], seseg[:, :si], axis=AX)
            rcp = statp.tile([P, 1], f32, tag="rcp")
            nc.vector.reciprocal(rcp[:], sumexp[:])

            # A^T via one XBAR DMA transpose: [128, W] -> [128, nk, 128].
            # Last tile via PE: no DMA latency on the end-of-kernel chain.
            attnT = attntp.tile([P, 16, P], bf16, tag="attnT")
            if j == 0:
                for c in range(nk):
                    tp = trp.tile([P, P], bf16, tag="tr")
                    nc.tensor.transpose(tp[:], attn[:, ts(c, P)], ident_b[:])
                    nc.vector.tensor_copy(attnT[:, c, :], tp[:])
            else:
                nc.scalar.dma_start(attnT[:, 0:nk, :], attn[:, 0:W],
                                    transpose=True)

            pp0 = ppp.tile([P, 512], f32, tag="pp", name="pp0")
            pp1 = ppp.tile([P, 512], f32, tag="pp", name="pp1")
            for c in range(nk):
                pos = c if c <= j else 8 + (c - j - 1)
                for dh, pp in ((0, pp0), (512, pp1)):
                    nc.tensor.matmul(
                        pp[:], attnT[:, c, :], xc_sb[:, pos, dh:dh + 512],
                        start=(c == 0), stop=(c == nk - 1))
            p_sb = workp.tile([P, 1024], bf16, tag="p")
            nc.vector.tensor_copy(p_sb[:, 0:512], pp0[:])
            nc.vector.tensor_copy(p_sb[:, 512:1024], pp1[:])

            if prev_o is not None:
                emit_o(*prev_o)
            prev_o = (j, p_sb, rcp)
        emit_o(*prev_o)
        trp.release()
        opsp.release()
        wvp.release()

    nc.compile()
    return nc


def _prep_inputs(sequence_repr, W_Q, W_K, W_V, mask):
    """Build the 8 per-core input dicts (host-side slicing/permutation)."""
    wqk = np.ascontiguousarray(W_Q @ W_K.T)
    wv_b = np.ascontiguousarray(W_V).astype(ml_dtypes.bfloat16)
    in_maps = []
    meta = []
    for c in range(NCORES):
        b, par = divmod(c, 2)
        qblocks = [2 * j + par for j in range(NJ)]
        oblocks = [2 * j + 1 - par for j in range(NJ)]
        posblocks = qblocks + oblocks
        rows_perm = np.concatenate(
            [np.arange(g * P, (g + 1) * P) for g in posblocks])
        qrows = rows_perm[:NJ * P]
        xb = sequence_repr[b]
        xct = np.ascontiguousarray(xb.T[:, rows_perm])
        xc = np.ascontiguousarray(xb[rows_perm]).astype(ml_dtypes.bfloat16)
        madd = np.empty((NJ * P, 2 * P), np.float32)
        for j in range(NJ):
            g = 2 * j + par
            gb = 2 * j + 1 - par
            qr = slice((2 * j + par) * P, (2 * j + par) * P + P)
            madd[j * P:(j + 1) * P, 0:P] = np.where(
                mask[b, qr, g * P:(g + 1) * P], 0.0, MASK_FILL)
            madd[j * P:(j + 1) * P, P:2 * P] = np.where(
                mask[b, qr, gb * P:(gb + 1) * P], 0.0, MASK_FILL)
        # pack [NJ*P, 2P] -> [P, NJ, 2P]  (madd_sb[p, j, :] = madd[j*P+p, :])
        madd_p = np.ascontiguousarray(
            madd.reshape(NJ, P, 2 * P).transpose(1, 0, 2)
        ).astype(ml_dtypes.bfloat16).reshape(P, NJ * 2 * P)
        in_maps.append({
            "xct": xct, "xc": xc,
            "wqk": wqk,
            "wv": wv_b,
            "madd": madd_p,
        })
        meta.append((b, qrows))
    return in_maps, meta


def run(sequence_repr, W_Q, W_K, W_V, mask, trace=False):
    from concourse.bass_utils import run_bass_kernel_spmd

    if "nc" not in _cache:
        _cache["nc"] = _build_program()
    nc = _cache["nc"]
    in_maps, meta = _prep_inputs(
        np.asarray(sequence_repr, np.float32), np.asarray(W_Q, np.float32),
        np.asarray(W_K, np.float32), np.asarray(W_V, np.float32),
        np.asarray(mask))
    res = run_bass_kernel_spmd(nc, in_maps, core_ids=list(range(NCORES)),
                               trace=trace)
    out = np.empty((B, S, D), np.float32)
    for c in range(NCORES):
        b, qrows = meta[c]
        out[b, qrows] = np.asarray(res.results[c]["out"]).astype(np.float32)
    return out, res


def kernel(**inputs):
    out, _ = run(**inputs)
    return out
